# revision 55
# baseline (speedup 1.0000x reference)
"""Trainium2 Bass kernel for nn_Attention_64819646431478.

Single-layer causal attention, B=1, T=2048, DIM=1024, 16 heads, head_dim=64,
f32, with RMSNorm (eps=f32 eps) on Q and K heads.

Sharding: tensor-parallel over heads across 8 NeuronCores (2 heads/core).
Each core computes its heads' Q/K/V projections, causal attention, and the
partial output projection against its 128-row slice of w_o; the host sums
the 8 partial outputs (the "all-reduce" of the hint, done at gather time).

Per-core kernel layout choices:
  - Host passes x pre-transposed (xT [C, T]) and per-core weight slices
    pre-transposed, so every matmul contraction dim is on partitions.
  - Matmul inputs are bf16 (1 cyc/row PE, pipelined LDWEIGHTS; fp32 is 4x
    slower, fp32r 2x + serial weight loads); accumulation stays f32 in PSUM.
  - Scores are computed transposed: ST[tk, tq] = K @ Q^T per head, so the
    softmax reduction (over tk) is a matmul-with-ones. Because Q/K are
    RMS-normalized, |scores| <= 8, so exp needs no max-subtraction.
  - The softmax denominator is obtained free by appending a ones column to
    V in the PV matmul (lhsT = [V_h | 1], m=65: row 64 accumulates sums).
  - Reciprocals use DVE reciprocal_approx_fast (~51 ULP, one pass; the
    exact DVE RECIPROCAL measures ~13us/row-tensor). NOTE: that custom DVE
    op silently corrupts at base partition != 0 - operands get own tiles.
  - w_o projection emitted transposed (out [m, t]); host transposes back.

v2 changes (161.9us -> ~133us):
  - All cross-engine chains are software-pipelined one chunk late, popped
    as PE fillers inside the next chunk's matmul stream: phase B defers
    each chunk's 1/rms-broadcast + Q/K normalize into the next chunk's
    projections; phase C defers each chunk's softmax-divide (b2) + w_o +
    output DMA into the next chunk's attention groups. PE never sits on a
    fresh ACT/DVE product, so HAM stays un-throttled (was 2x-clocked 27%
    of the span at the old chunk boundaries).
  - The causal diagonal band is ragged: r-tile 4c+s only computes
    tq >= 128s (widths 512/384/256/128 packed into 1.25 score tiles), and
    only the four 128x128 true-diagonal blocks get a triangle mask. Cuts
    ~1M wasted score elements per head-chunk from PE, exp (ACT), and the
    full-width mask multiplies (DVE).
  - Output partials are bf16 (host accumulates in f32): halves the output
    DMA; adds ~3e-4 scaled error.
  - x loads are chunk-major and split across two DMA queues, so chunk 0's
    projections start ~8us earlier.
"""

import os
import sys
import types

import numpy as np

# --- environment bootstrap (harness may run us from a bare directory) ---
for _p in ("/root/.axon_site", "/root/.axon_site/_ro/trn_rl_repo",
           "/root/.axon_site/_ro/pypackages", "/opt/trn_rl_repo"):
    if os.path.isdir(_p) and _p not in sys.path:
        sys.path.append(_p)


def _install_ntff_shim():
    """Provide antenv.axon_hooks (missing in this image) so trace=True works."""
    if "antenv.axon_hooks" in sys.modules:
        return
    mod = types.ModuleType("antenv.axon_hooks")
    mod._hook = None
    mod.set_axon_ntff_profile_hook = lambda h: setattr(mod, "_hook", h)
    mod.get_axon_ntff_profile_hook = lambda: mod._hook
    sys.modules["antenv.axon_hooks"] = mod
    try:
        import antenv
        antenv.axon_hooks = mod
        from trn_agent_boot.trn_boot import _ntff_profile_via_ctypes
        mod.set_axon_ntff_profile_hook(
            _ntff_profile_via_ctypes("/opt/axon/libaxon_pjrt.so"))
    except Exception:
        pass


_install_ntff_shim()

import ml_dtypes  # noqa: E402

import concourse.mybir as mybir  # noqa: E402
import concourse.tile as tile  # noqa: E402
from concourse import bacc  # noqa: E402

F32 = mybir.dt.float32
BF16 = mybir.dt.bfloat16
NP_BF16 = ml_dtypes.bfloat16
AF = mybir.ActivationFunctionType

_TABLES_PATCHED = False


def _pin_act_table_set():
    """Make the ACT table-load chooser resolve Exp/Ln/Square/Copy to the one
    set that holds all four (natural_log_exp_and_others). The default
    per-function primary sets differ (exp_and_others vs natural_log vs
    sqrt_and_others), so a kernel mixing them reloads tables mid-kernel
    (~2.7us each) - and the reload lands exactly at the projection ->
    attention boundary, where the resulting PE idle re-throttles HAM to
    1.2GHz for ~20us. Stripping these funcs from every other set - order
    and indices preserved, so walrus' id mapping stays valid - leaves the
    chooser exactly one set and the kernel loads tables once."""
    global _TABLES_PATCHED
    if _TABLES_PATCHED:
        return
    import functools

    import concourse.bacc as bacc_mod
    from concourse.hw_specs import get_activation_tables as _orig

    keep = {AF.Exp, AF.Ln, AF.Square, AF.Copy}
    target = "natural_log_exp_and_others"

    @functools.lru_cache(maxsize=None)
    def patched(arch):
        tabs = _orig(arch)
        if target not in tabs or not keep.issubset(tabs[target]):
            return tabs
        return {name: (funcs if name == target else funcs - keep)
                for name, funcs in tabs.items()}

    bacc_mod.get_activation_tables = patched
    _TABLES_PATCHED = True

T = 2048
C = 1024
D = 64
NCORES = 8
HPC = 2            # heads per core
JPC = HPC * D      # 128 j-columns per core
NTQ = 4            # tq chunks of 512
TQ = 512
NTK = 16           # tk tiles of 128
EPS = float(np.finfo(np.float32).eps)


def build_nc():
    _pin_act_table_set()
    nc = bacc.Bacc("TRN2", target_bir_lowering=False, debug=False,
                   num_devices=NCORES)

    xT_d = nc.dram_tensor("xT", [C, T], BF16, kind="ExternalInput")
    wqkv_d = nc.dram_tensor("wqkv", [C, 3 * JPC], BF16, kind="ExternalInput")
    wo_d = nc.dram_tensor("wo", [JPC, C], BF16, kind="ExternalInput")
    masks_d = nc.dram_tensor("masks", [4, 128, TQ], BF16, kind="ExternalInput")
    gq_d = nc.dram_tensor("gq", [2, 128], BF16, kind="ExternalInput")
    gk_d = nc.dram_tensor("gk", [2, 128], BF16, kind="ExternalInput")
    ones2_d = nc.dram_tensor("ones2", [2, 128], BF16, kind="ExternalInput")
    onescol_d = nc.dram_tensor("onescol", [128, 2], BF16, kind="ExternalInput")
    ident_d = nc.dram_tensor("ident", [128, 128], BF16, kind="ExternalInput")
    vones_d = nc.dram_tensor("vones", [128, 32], BF16, kind="ExternalInput")
    outT_d = nc.dram_tensor("outT", [C, T], BF16, kind="ExternalOutput")

    with tile.TileContext(nc) as tc, nc.allow_low_precision("bf16 kernel"):
        from contextlib import ExitStack
        with ExitStack() as ctx:
            consts = ctx.enter_context(tc.tile_pool(name="consts", bufs=1))
            acts = ctx.enter_context(tc.tile_pool(name="acts", bufs=1))

            # ---- constants / inputs to SBUF ----
            wsb = consts.tile([128, 8, 3 * JPC], BF16)
            nc.gpsimd.dma_start(
                out=wsb[:], in_=wqkv_d.rearrange("(c p) j -> p c j", p=128))
            wo_sb = consts.tile([128, C], BF16)
            nc.sync.dma_start(out=wo_sb[:], in_=wo_d[:])
            msb = consts.tile([128, 4, TQ], BF16)
            nc.sync.dma_start(out=msb[:],
                              in_=masks_d.rearrange("s p f -> p s f"))
            gq_sb = consts.tile([2, 128], BF16)
            nc.gpsimd.dma_start(out=gq_sb[:], in_=gq_d[:])
            gk_sb = consts.tile([2, 128], BF16)
            nc.gpsimd.dma_start(out=gk_sb[:], in_=gk_d[:])
            oneh_sb = []
            for h in range(HPC):
                t_ = consts.tile([1, 128], BF16, name=f"oneh{h}")
                nc.gpsimd.dma_start(out=t_[:], in_=ones2_d[h:h + 1, :])
                oneh_sb.append(t_)
            onescol_sb = consts.tile([128, 2], BF16)
            nc.gpsimd.dma_start(out=onescol_sb[:], in_=onescol_d[:])
            ident_sb = consts.tile([128, 128], BF16)
            nc.sync.dma_start(out=ident_sb[:], in_=ident_d[:])

            # ---- persistent activations ----
            QTn = acts.tile([128, T], BF16)     # [ (h,d), t ] normalized Q^T
            KTn = acts.tile([128, T], BF16)
            V_sb = acts.tile([128, NTK, 130], BF16)  # [tk, r, (Vh0|1|Vh1|1)]
            ctx_un = acts.tile([128, T], BF16)  # unnormalized ctx^T
            ctxT = acts.tile([128, T], BF16)    # normalized ctx^T
            # NOTE: reciprocal_approx_fast (custom DVE op) only works at
            # base partition 0 -> every operand gets its own tile.
            rms_q = acts.tile([2, T], F32)
            rms_k = acts.tile([2, T], F32)
            rrf_q = acts.tile([2, T], F32)      # 1/rms (fp32, approx recip)
            rrf_k = acts.tile([2, T], F32)
            rec_q = acts.tile([2, T], BF16)     # rsqrt(mean q^2 + eps)
            rec_k = acts.tile([2, T], BF16)
            sg = [acts.tile([1, T], F32, name=f"sg{h}") for h in range(HPC)]
            sgf = [acts.tile([1, T], F32, name=f"sgf{h}") for h in range(HPC)]
            recs = [acts.tile([1, T], BF16, name=f"recs{h}")
                    for h in range(HPC)]

            # ones columns of V tiles (cols 64 and 129)
            vones_r = vones_d.rearrange("p (r u) -> p r u", u=2)
            nc.sync.dma_start(out=V_sb[:, :, 64:65], in_=vones_r[:, :, 0:1])
            nc.sync.dma_start(out=V_sb[:, :, 129:130],
                              in_=vones_r[:, :, 1:2])
            eps_sb = consts.tile([2, 1], F32)
            nc.vector.memset(eps_sb[:], EPS)

            # ================= Phase B: projections + RMSNorm ==============
            with (
                tc.tile_pool(name="xtp", bufs=1) as xtp,
                tc.tile_pool(name="sqp", bufs=3) as sqp,
                tc.tile_pool(name="ps_proj", bufs=3, space="PSUM") as ps_proj,
                tc.tile_pool(name="ps_sums", bufs=1, space="PSUM") as ps_sums,
                tc.tile_pool(name="ps_b", bufs=2, space="PSUM") as ps_b,
                tc.tile_pool(name="ps_tp", bufs=2, space="PSUM") as ps_tp,
            ):
                xT_sb = xtp.tile([128, 8, T], BF16)
                xT_r = xT_d.rearrange("(c p) t -> p c t", p=128)
                # chunk-major loads split over two DMA queues, so chunk 0's
                # projections start ~8us earlier than with per-ci full rows
                for c4 in range(NTQ):
                    sl = slice(TQ * c4, TQ * (c4 + 1))
                    for ci in range(8):
                        eng = nc.gpsimd if ci % 2 == 0 else nc.sync
                        eng.dma_start(out=xT_sb[:, ci, sl],
                                      in_=xT_r[:, ci, sl])
                VT_sb = xtp.tile([128, T], BF16)   # [ j, t ] V^T
                QT_raw = xtp.tile([128, T], BF16)  # un-normalized Q^T
                KT_raw = xtp.tile([128, T], BF16)

                def mk_bb(g2_sb, rec_sb, raw, dst, sl):
                    # chunk c's 1/rms broadcast + normalize, deferred into
                    # chunk c+1's projections (the Sqrt/recip chain gets a
                    # whole proj group to drain -> no PE stall)
                    def f():
                        bb = ps_b.tile([128, TQ], F32, tag="b", name="bb")
                        nc.tensor.matmul(bb[:], g2_sb[:], rec_sb[:, sl],
                                         start=True, stop=True)
                        nc.vector.tensor_mul(dst[:, sl], raw[:, sl], bb[:])
                    return f

                # Q/K/V per tq-chunk so attention can start on early chunks.
                # rsqrt = approx-recip(Sqrt): Square/Sqrt share one ACT
                # table set, reciprocal_approx_fast is one DVE pass.
                # Every PE op that consumes a fresh ACT/DVE product (sums
                # after Square, transposes after the VT copy, bb after the
                # recip chain) is deferred >=1 proj-group behind its
                # producer, so phase B has no PE micro-gaps and HAM warms
                # at ~10us instead of ~28us.
                def mk_sums(sq, g2_sb, rms_sb, rrf_sb, rec_sb, raw, dst, sl):
                    # rec = (mean+eps)^-1/2 = exp(-0.5*ln(mean+eps)): stays
                    # inside the pinned exp/ln/square table set (Sqrt would
                    # force a mid-kernel table reload) and needs no DVE pass
                    def f():
                        sums = ps_sums.tile([2, TQ], F32, tag="sums",
                                            name="sums")
                        nc.tensor.matmul(sums[:], onescol_sb[:], sq[:],
                                         start=True, stop=True)
                        nc.scalar.activation(rms_sb[:, sl], sums[:],
                                             AF.Ln, bias=eps_sb[:],
                                             scale=1.0 / D)
                        nc.scalar.activation(rec_sb[:, sl], rms_sb[:, sl],
                                             AF.Exp, scale=-0.5)
                    return f

                def mk_tp(c4):
                    def f():
                        for r in range(4 * c4, 4 * c4 + 4):
                            tp = ps_tp.tile([128, 128], BF16, tag="tp",
                                            name=f"tp{r}")
                            nc.tensor.transpose(
                                tp[:], VT_sb[:, 128 * r:128 * (r + 1)],
                                ident_sb[:])
                            nc.vector.tensor_copy(V_sb[:, r, 0:64],
                                                  tp[:, 0:64])
                            nc.vector.tensor_copy(V_sb[:, r, 65:129],
                                                  tp[:, 64:128])
                    return f

                pend_b = []
                for c4 in range(NTQ):
                    sl = slice(TQ * c4, TQ * (c4 + 1))
                    chains = []
                    for jbase, g2_sb, rms_sb, rrf_sb, rec_sb, raw, dst in (
                        (0, gq_sb, rms_q, rrf_q, rec_q, QT_raw, QTn),
                        (JPC, gk_sb, rms_k, rrf_k, rec_k, KT_raw, KTn),
                    ):
                        pp = ps_proj.tile([128, TQ], F32, tag="proj",
                                          name="pp")
                        for ci in range(8):
                            nc.tensor.matmul(
                                pp[:], wsb[:, ci, jbase:jbase + 128],
                                xT_sb[:, ci, sl],
                                start=(ci == 0), stop=(ci == 7))
                        sq = sqp.tile([128, TQ], BF16, tag="sq", name="sq")
                        nc.scalar.activation(sq[:], pp[:], AF.Square)
                        nc.vector.tensor_copy(raw[:, sl], pp[:])
                        if jbase == 0:  # after Q proj: drain prev chunk
                            while pend_b:
                                pend_b.pop(0)()
                        else:           # after K proj: this chunk's Q sums
                            chains[0]()
                        chains.append(
                            mk_sums(sq, g2_sb, rms_sb, rrf_sb, rec_sb,
                                    raw, dst, sl))
                        pend_b.append(
                            mk_bb(g2_sb, rec_sb, raw, dst, sl))

                    # V^T projection chunk
                    pv = ps_proj.tile([128, TQ], F32, tag="proj", name="pv")
                    for ci in range(8):
                        nc.tensor.matmul(
                            pv[:], wsb[:, ci, 2 * JPC:3 * JPC],
                            xT_sb[:, ci, sl],
                            start=(ci == 0), stop=(ci == 7))
                    chains[1]()   # this chunk's K sums after V proj
                    nc.vector.tensor_copy(VT_sb[:, sl], pv[:])
                    pend_b.append(mk_tp(c4))
                while pend_b:
                    pend_b.pop(0)()

            # ========== Phase C: attention + normalize + w_o, per chunk ====
            outT_r = outT_d.rearrange("(m p) t -> p m t", p=128)
            with (
                tc.tile_pool(name="ep", bufs=6) as ep,
                tc.tile_pool(name="stgp", bufs=1) as stgp,
                tc.tile_pool(name="ps_st0", bufs=1, space="PSUM") as ps_st0,
                tc.tile_pool(name="ps_st1", bufs=1, space="PSUM") as ps_st1,
                tc.tile_pool(name="ps_ot", bufs=1, space="PSUM") as ps_ot,
                tc.tile_pool(name="ps_wrk", bufs=2, space="PSUM") as ps_wrk,
            ):
                st_pools = (ps_st0, ps_st1)
                stg_big = stgp.tile([128, 8, T], BF16)  # w_o out staging

                def mk_pending(p4):
                    """Chunk p4's normalize + w_o, deferred into the next
                    chunk's attention loop as PE fillers: its DVE reciprocal
                    chain gets the chunk boundary to drain, so the PE never
                    waits on it (the b2/w_o stall here re-throttled HAM to
                    1.2GHz for ~17us in the undeferred layout)."""
                    psl = slice(TQ * p4, TQ * (p4 + 1))
                    todo = []

                    def b2fn(h, psl=psl):
                        hsl = slice(64 * h, 64 * (h + 1))
                        b2 = ps_wrk.tile([128, TQ], F32, tag="wrk",
                                         name=f"b2{h}_{p4}")
                        nc.tensor.matmul(b2[:], oneh_sb[h][:],
                                         recs[h][0:1, psl],
                                         start=True, stop=True)
                        nc.vector.tensor_mul(ctxT[hsl, psl],
                                             ctx_un[hsl, psl], b2[hsl, :])

                    def wofn(mu, psl=psl, p4=p4):
                        wop = ps_wrk.tile([128, TQ], F32, tag="wrk",
                                          name=f"wop{mu}_{p4}")
                        nc.tensor.matmul(wop[:],
                                         wo_sb[:, 128 * mu:128 * (mu + 1)],
                                         ctxT[:, psl], start=True, stop=True)
                        nc.vector.tensor_copy(stg_big[:, mu, psl], wop[:])
                        nc.sync.dma_start(out=outT_r[:, mu, psl],
                                          in_=stg_big[:, mu, psl])

                    for h in range(HPC):
                        todo.append(lambda h=h: b2fn(h))
                    for mu in range(8):
                        todo.append(lambda mu=mu: wofn(mu))
                    return todo

                pending = []
                for c4 in range(NTQ):
                    qsl = slice(TQ * c4, TQ * (c4 + 1))
                    n_tk = 4 * (c4 + 1)
                    ot = [ps_ot.tile([65, TQ], F32, tag=f"ot{h}",
                                     name=f"ot{h}_{c4}")
                          for h in range(HPC)]
                    # PV matmuls are deferred by one group (popped from
                    # pvq), so each exp has a full group of score matmuls
                    # to complete -> the PE never waits on the ACT stream
                    pvq = []

                    def mk_pv(h, r0, r1, e_t):
                        def f():
                            for rl, r in ((0, r0), (1, r1)):
                                nc.tensor.matmul(
                                    ot[h][:],
                                    V_sb[:, r, 65 * h:65 * (h + 1)],
                                    e_t[:, TQ * rl:TQ * (rl + 1)],
                                    start=(r == 0), stop=False)
                        return f

                    for g in range(2 * c4):   # off-diagonal r-tiles, paired
                        for h in range(HPC):
                            hsl = slice(64 * h, 64 * (h + 1))
                            st = st_pools[h].tile([128, 2 * TQ], F32,
                                                  tag=f"st{h}",
                                                  name=f"st{h}")
                            for rl in range(2):
                                r = 2 * g + rl
                                nc.tensor.matmul(
                                    st[:, TQ * rl:TQ * (rl + 1)],
                                    KTn[hsl, 128 * r:128 * (r + 1)],
                                    QTn[hsl, qsl], start=True, stop=True)
                            e_t = ep.tile([128, 2 * TQ], BF16, tag=f"e{h}",
                                          name=f"e{h}")
                            nc.scalar.activation(e_t[:], st[:], AF.Exp,
                                                 scale=float(D) ** -0.5)
                            pvq.append(mk_pv(h, 2 * g, 2 * g + 1, e_t))
                        while len(pvq) > 2:
                            pvq.pop(0)()
                        if pending:
                            pending.pop(0)()
                    # ragged diagonal band: r-tile 4*c4+s only covers
                    # tq>=128*s (widths 512/384/256/128); only the four
                    # 128x128 true-diagonal blocks need a triangle mask
                    # (= first 128 cols of msb plane 0)
                    tri = msb[:, 0, 0:128]
                    for h in range(HPC):
                        hsl = slice(64 * h, 64 * (h + 1))
                        r = 4 * c4

                        def kt(s, hsl=hsl, r=r):
                            return KTn[hsl, 128 * (r + s):128 * (r + s + 1)]

                        def qt(s, hsl=hsl, c4=c4):
                            return QTn[hsl, TQ * c4 + 128 * s:TQ * (c4 + 1)]

                        stb = st_pools[h].tile([128, 2 * TQ], F32,
                                               tag=f"st{h}", name=f"stb{h}")
                        sb2 = ps_wrk.tile([128, TQ], F32, tag="wrk",
                                          name=f"sb2{h}")
                        nc.tensor.matmul(stb[:, 0:512], kt(0), qt(0),
                                         start=True, stop=True)
                        nc.tensor.matmul(stb[:, 512:896], kt(1), qt(1),
                                         start=True, stop=True)
                        nc.tensor.matmul(stb[:, 896:1024], kt(3), qt(3),
                                         start=True, stop=True)
                        nc.tensor.matmul(sb2[:, 0:256], kt(2), qt(2),
                                         start=True, stop=True)
                        e1 = ep.tile([128, 2 * TQ], BF16, tag=f"e{h}",
                                     name=f"eb{h}")
                        nc.scalar.activation(e1[:], stb[:], AF.Exp,
                                             scale=float(D) ** -0.5)
                        e2 = ep.tile([128, 256], BF16, tag=f"e2{h}",
                                     name=f"eb2{h}")
                        nc.scalar.activation(e2[:], sb2[:, 0:256], AF.Exp,
                                             scale=float(D) ** -0.5)
                        for blk in (e1[:, 0:128], e1[:, 512:640],
                                    e1[:, 896:1024], e2[:, 0:128]):
                            nc.vector.tensor_mul(blk, blk, tri)

                        def bandpv(h=h, r=r, e1=e1, e2=e2):
                            vs = V_sb[:, :, 65 * h:65 * (h + 1)]
                            nc.tensor.matmul(ot[h][:, 0:512], vs[:, r],
                                             e1[:, 0:512],
                                             start=(r == 0), stop=False)
                            nc.tensor.matmul(ot[h][:, 128:512], vs[:, r + 1],
                                             e1[:, 512:896],
                                             start=False, stop=False)
                            nc.tensor.matmul(ot[h][:, 256:512], vs[:, r + 2],
                                             e2[:, 0:256],
                                             start=False, stop=False)
                            nc.tensor.matmul(ot[h][:, 384:512], vs[:, r + 3],
                                             e1[:, 896:1024],
                                             start=False, stop=True)

                        pvq.append(bandpv)
                        while len(pvq) > 2:
                            pvq.pop(0)()
                        if pending:
                            pending.pop(0)()
                    while pvq:
                        pvq.pop(0)()
                    while pending:
                        pending.pop(0)()
                    # stage ctx + softmax sums (approx recip); b2/w_o are
                    # deferred into the next chunk's attention
                    for h in range(HPC):
                        hsl = slice(64 * h, 64 * (h + 1))
                        nc.vector.tensor_copy(ctx_un[hsl, qsl],
                                              ot[h][0:64, :])
                        nc.vector.tensor_copy(sg[h][0:1, qsl],
                                              ot[h][64:65, :])
                        nc.vector.reciprocal_approx_fast(
                            out=sgf[h][0:1, qsl], in_=sg[h][0:1, qsl])
                        nc.vector.tensor_copy(recs[h][0:1, qsl],
                                              sgf[h][0:1, qsl])
                    pending = mk_pending(c4)
                while pending:
                    pending.pop(0)()

    nc.compile()
    return nc


_NC_CACHE = None


def _get_nc():
    global _NC_CACHE
    if _NC_CACHE is None:
        _NC_CACHE = build_nc()
    return _NC_CACHE


def _make_in_maps(x, w_q, w_k, w_v, w_o, q_gamma, k_gamma):
    x = np.asarray(x, dtype=np.float32)
    xT = np.ascontiguousarray(x.reshape(T, C).T).astype(NP_BF16)  # [C, T]

    p = np.arange(128)
    f = np.arange(TQ)
    masks = np.zeros((4, 128, TQ), dtype=NP_BF16)
    for s in range(4):
        masks[s] = (f[None, :] >= (p[:, None] + 128 * s)).astype(NP_BF16)

    blk = (p[None, :] // 64 == np.arange(2)[:, None])      # [2, 128] bool
    gq = blk * np.tile(np.asarray(q_gamma, np.float32), 2)[None, :]
    gk = blk * np.tile(np.asarray(k_gamma, np.float32), 2)[None, :]
    ones2 = blk.astype(NP_BF16)
    onescol = np.ascontiguousarray(ones2.T)
    ident = np.eye(128, dtype=NP_BF16)

    common = dict(xT=xT, masks=masks,
                  gq=gq.astype(NP_BF16), gk=gk.astype(NP_BF16),
                  ones2=ones2, onescol=onescol, ident=ident,
                  vones=np.ones((128, 32), dtype=NP_BF16))

    in_maps = []
    for i in range(NCORES):
        rows = slice(JPC * i, JPC * (i + 1))
        wqkv = np.concatenate(
            [np.asarray(w_q, np.float32)[rows].T,
             np.asarray(w_k, np.float32)[rows].T,
             np.asarray(w_v, np.float32)[rows].T], axis=1)  # [C, 384]
        wo = np.asarray(w_o, np.float32)[:, rows].T          # [128, C]
        in_maps.append(dict(common,
                            wqkv=np.ascontiguousarray(wqkv).astype(NP_BF16),
                            wo=np.ascontiguousarray(wo).astype(NP_BF16)))
    return in_maps


def _run(x, w_q, w_k, w_v, w_o, q_gamma, k_gamma, trace=False):
    import time

    from concourse.bass_utils import run_bass_kernel_spmd
    nc = _get_nc()
    in_maps = _make_in_maps(x, w_q, w_k, w_v, w_o, q_gamma, k_gamma)
    res = None
    for attempt in range(3):
        try:
            res = run_bass_kernel_spmd(nc, in_maps, list(range(NCORES)),
                                       trace=trace)
            break
        except Exception:
            # rare transient NRT_EXEC_UNIT_UNRECOVERABLE under axon; the
            # terminal resets the device on the next load
            if attempt == 2:
                raise
            time.sleep(3.0)
    acc = np.zeros((C, T), dtype=np.float64)
    for r in res.results:
        acc += r["outT"].astype(np.float64)
    out = acc.T.astype(np.float32).reshape(1, T, C)
    return out, res


def kernel(x, w_q, w_k, w_v, w_o, q_gamma, k_gamma):
    out, _ = _run(x, w_q, w_k, w_v, w_o, q_gamma, k_gamma, trace=False)
    return out



# revision 57
# speedup vs baseline: 1.0587x; 1.0587x over previous
"""Trainium2 Bass kernel for nn_Attention_64819646431478.

Single-layer causal attention, B=1, T=2048, DIM=1024, 16 heads, head_dim=64,
f32, with RMSNorm (eps=f32 eps) on Q and K heads.

Sharding: tensor-parallel over heads across 8 NeuronCores (2 heads/core).
Each core computes its heads' Q/K/V projections, causal attention, and the
partial output projection against its 128-row slice of w_o; the host sums
the 8 partial outputs (the "all-reduce" of the hint, done at gather time).

Per-core kernel layout choices:
  - Host passes x pre-transposed (xT [C, T]) and per-core weight slices
    pre-transposed, so every matmul contraction dim is on partitions.
  - Matmul inputs are bf16 (1 cyc/row PE, pipelined LDWEIGHTS; fp32 is 4x
    slower, fp32r 2x + serial weight loads); accumulation stays f32 in PSUM.
  - Scores are computed transposed: ST[tk, tq] = K @ Q^T per head, so the
    softmax reduction (over tk) is a matmul-with-ones. Because Q/K are
    RMS-normalized, |scores| <= 8, so exp needs no max-subtraction.
  - The softmax denominator is obtained free by appending a ones column to
    V in the PV matmul (lhsT = [V_h | 1], m=65: row 64 accumulates sums).
  - Reciprocals use DVE reciprocal_approx_fast (~51 ULP, one pass; the
    exact DVE RECIPROCAL measures ~13us/row-tensor). NOTE: that custom DVE
    op silently corrupts at base partition != 0 - operands get own tiles.
  - w_o projection emitted transposed (out [m, t]); host transposes back.

v2 changes (161.9us -> ~133us):
  - All cross-engine chains are software-pipelined one chunk late, popped
    as PE fillers inside the next chunk's matmul stream: phase B defers
    each chunk's 1/rms-broadcast + Q/K normalize into the next chunk's
    projections; phase C defers each chunk's softmax-divide (b2) + w_o +
    output DMA into the next chunk's attention groups. PE never sits on a
    fresh ACT/DVE product, so HAM stays un-throttled (was 2x-clocked 27%
    of the span at the old chunk boundaries).
  - The causal diagonal band is ragged: r-tile 4c+s only computes
    tq >= 128s (widths 512/384/256/128 packed into 1.25 score tiles), and
    only the four 128x128 true-diagonal blocks get a triangle mask. Cuts
    ~1M wasted score elements per head-chunk from PE, exp (ACT), and the
    full-width mask multiplies (DVE).
  - Output partials are bf16 (host accumulates in f32): halves the output
    DMA; adds ~3e-4 scaled error.
  - x loads are chunk-major and split across two DMA queues, so chunk 0's
    projections start ~8us earlier.
"""

import os
import sys
import types

import numpy as np

# --- environment bootstrap (harness may run us from a bare directory) ---
for _p in ("/root/.axon_site", "/root/.axon_site/_ro/trn_rl_repo",
           "/root/.axon_site/_ro/pypackages", "/opt/trn_rl_repo"):
    if os.path.isdir(_p) and _p not in sys.path:
        sys.path.append(_p)


def _install_ntff_shim():
    """Provide antenv.axon_hooks (missing in this image) so trace=True works."""
    if "antenv.axon_hooks" in sys.modules:
        return
    mod = types.ModuleType("antenv.axon_hooks")
    mod._hook = None
    mod.set_axon_ntff_profile_hook = lambda h: setattr(mod, "_hook", h)
    mod.get_axon_ntff_profile_hook = lambda: mod._hook
    sys.modules["antenv.axon_hooks"] = mod
    try:
        import antenv
        antenv.axon_hooks = mod
        from trn_agent_boot.trn_boot import _ntff_profile_via_ctypes
        mod.set_axon_ntff_profile_hook(
            _ntff_profile_via_ctypes("/opt/axon/libaxon_pjrt.so"))
    except Exception:
        pass


_install_ntff_shim()

import ml_dtypes  # noqa: E402

import concourse.mybir as mybir  # noqa: E402
import concourse.tile as tile  # noqa: E402
from concourse import bacc  # noqa: E402

F32 = mybir.dt.float32
BF16 = mybir.dt.bfloat16
NP_BF16 = ml_dtypes.bfloat16
AF = mybir.ActivationFunctionType

_TABLES_PATCHED = False


def _pin_act_table_set():
    """Make the ACT table-load chooser resolve Exp/Ln/Square/Copy to the one
    set that holds all four (natural_log_exp_and_others). The default
    per-function primary sets differ (exp_and_others vs natural_log vs
    sqrt_and_others), so a kernel mixing them reloads tables mid-kernel
    (~2.7us each) - and the reload lands exactly at the projection ->
    attention boundary, where the resulting PE idle re-throttles HAM to
    1.2GHz for ~20us. Stripping these funcs from every other set - order
    and indices preserved, so walrus' id mapping stays valid - leaves the
    chooser exactly one set and the kernel loads tables once."""
    global _TABLES_PATCHED
    if _TABLES_PATCHED:
        return
    import functools

    import concourse.bacc as bacc_mod
    from concourse.hw_specs import get_activation_tables as _orig

    keep = {AF.Exp, AF.Ln, AF.Square, AF.Copy}
    target = "natural_log_exp_and_others"

    @functools.lru_cache(maxsize=None)
    def patched(arch):
        tabs = _orig(arch)
        if target not in tabs or not keep.issubset(tabs[target]):
            return tabs
        return {name: (funcs if name == target else funcs - keep)
                for name, funcs in tabs.items()}

    bacc_mod.get_activation_tables = patched
    _TABLES_PATCHED = True

T = 2048
C = 1024
D = 64
NCORES = 8
HPC = 2            # heads per core
JPC = HPC * D      # 128 j-columns per core
NTQ = 4            # tq chunks of 512
TQ = 512
NTK = 16           # tk tiles of 128
EPS = float(np.finfo(np.float32).eps)


def build_nc():
    _pin_act_table_set()
    nc = bacc.Bacc("TRN2", target_bir_lowering=False, debug=False,
                   num_devices=NCORES)

    xT_d = nc.dram_tensor("xT", [C, T], BF16, kind="ExternalInput")
    wqkv_d = nc.dram_tensor("wqkv", [C, 3 * JPC], BF16, kind="ExternalInput")
    wo_d = nc.dram_tensor("wo", [JPC, C], BF16, kind="ExternalInput")
    masks_d = nc.dram_tensor("masks", [4, 128, TQ], BF16, kind="ExternalInput")
    gq_d = nc.dram_tensor("gq", [2, 128], BF16, kind="ExternalInput")
    gk_d = nc.dram_tensor("gk", [2, 128], BF16, kind="ExternalInput")
    ones2_d = nc.dram_tensor("ones2", [2, 128], BF16, kind="ExternalInput")
    onescol_d = nc.dram_tensor("onescol", [128, 2], BF16, kind="ExternalInput")
    ident_d = nc.dram_tensor("ident", [128, 128], BF16, kind="ExternalInput")
    vones_d = nc.dram_tensor("vones", [128, 32], BF16, kind="ExternalInput")
    outT_d = nc.dram_tensor("outT", [C, T], BF16, kind="ExternalOutput")

    with tile.TileContext(nc) as tc, nc.allow_low_precision("bf16 kernel"):
        from contextlib import ExitStack
        with ExitStack() as ctx:
            consts = ctx.enter_context(tc.tile_pool(name="consts", bufs=1))
            acts = ctx.enter_context(tc.tile_pool(name="acts", bufs=1))

            # ---- constants / inputs to SBUF ----
            wsb = consts.tile([128, 8, 3 * JPC], BF16)
            nc.gpsimd.dma_start(
                out=wsb[:], in_=wqkv_d.rearrange("(c p) j -> p c j", p=128))
            wo_sb = consts.tile([128, C], BF16)
            nc.sync.dma_start(out=wo_sb[:], in_=wo_d[:])
            msb = consts.tile([128, 4, TQ], BF16)
            nc.sync.dma_start(out=msb[:],
                              in_=masks_d.rearrange("s p f -> p s f"))
            gq_sb = consts.tile([2, 128], BF16)
            nc.gpsimd.dma_start(out=gq_sb[:], in_=gq_d[:])
            gk_sb = consts.tile([2, 128], BF16)
            nc.gpsimd.dma_start(out=gk_sb[:], in_=gk_d[:])
            oneh_sb = []
            for h in range(HPC):
                t_ = consts.tile([1, 128], BF16, name=f"oneh{h}")
                nc.gpsimd.dma_start(out=t_[:], in_=ones2_d[h:h + 1, :])
                oneh_sb.append(t_)
            onescol_sb = consts.tile([128, 2], BF16)
            nc.gpsimd.dma_start(out=onescol_sb[:], in_=onescol_d[:])
            ident_sb = consts.tile([128, 128], BF16)
            nc.sync.dma_start(out=ident_sb[:], in_=ident_d[:])

            # ---- persistent activations ----
            QTn = acts.tile([128, T], BF16)     # [ (h,d), t ] normalized Q^T
            KTn = acts.tile([128, T], BF16)
            V_sb = acts.tile([128, NTK, 130], BF16)  # [tk, r, (Vh0|1|Vh1|1)]
            ctx_un = acts.tile([128, T], BF16)  # unnormalized ctx^T
            ctxT = acts.tile([128, T], BF16)    # normalized ctx^T
            # NOTE: reciprocal_approx_fast (custom DVE op) only works at
            # base partition 0 -> every operand gets its own tile.
            rms_q = acts.tile([2, T], F32)
            rms_k = acts.tile([2, T], F32)
            rrf_q = acts.tile([2, T], F32)      # 1/rms (fp32, approx recip)
            rrf_k = acts.tile([2, T], F32)
            rec_q = acts.tile([2, T], BF16)     # rsqrt(mean q^2 + eps)
            rec_k = acts.tile([2, T], BF16)
            sg = [acts.tile([1, T], F32, name=f"sg{h}") for h in range(HPC)]
            sgf = [acts.tile([1, T], F32, name=f"sgf{h}") for h in range(HPC)]
            recs = [acts.tile([1, T], BF16, name=f"recs{h}")
                    for h in range(HPC)]

            # ones columns of V tiles (cols 64 and 129)
            vones_r = vones_d.rearrange("p (r u) -> p r u", u=2)
            nc.sync.dma_start(out=V_sb[:, :, 64:65], in_=vones_r[:, :, 0:1])
            nc.sync.dma_start(out=V_sb[:, :, 129:130],
                              in_=vones_r[:, :, 1:2])
            eps_sb = consts.tile([2, 1], F32)
            nc.vector.memset(eps_sb[:], EPS)

            # ================= Phase B: projections + RMSNorm ==============
            with (
                tc.tile_pool(name="xtp", bufs=1) as xtp,
                tc.tile_pool(name="sqp", bufs=3) as sqp,
                tc.tile_pool(name="ps_proj", bufs=3, space="PSUM") as ps_proj,
                tc.tile_pool(name="ps_sums", bufs=1, space="PSUM") as ps_sums,
                tc.tile_pool(name="ps_b", bufs=2, space="PSUM") as ps_b,
                tc.tile_pool(name="ps_tp", bufs=2, space="PSUM") as ps_tp,
            ):
                xT_sb = xtp.tile([128, 8, T], BF16)
                xT_r = xT_d.rearrange("(c p) t -> p c t", p=128)
                # chunk-major loads split over two DMA queues, so chunk 0's
                # projections start ~8us earlier than with per-ci full rows
                for c4 in range(NTQ):
                    sl = slice(TQ * c4, TQ * (c4 + 1))
                    for ci in range(8):
                        eng = nc.gpsimd if ci % 2 == 0 else nc.sync
                        eng.dma_start(out=xT_sb[:, ci, sl],
                                      in_=xT_r[:, ci, sl])
                VT_sb = xtp.tile([128, T], BF16)   # [ j, t ] V^T
                QT_raw = xtp.tile([128, T], BF16)  # un-normalized Q^T
                KT_raw = xtp.tile([128, T], BF16)

                def mk_bb(g2_sb, rec_sb, raw, dst, sl):
                    # chunk c's 1/rms broadcast + normalize, deferred into
                    # chunk c+1's projections (the Sqrt/recip chain gets a
                    # whole proj group to drain -> no PE stall)
                    def f():
                        bb = ps_b.tile([128, TQ], F32, tag="b", name="bb")
                        nc.tensor.matmul(bb[:], g2_sb[:], rec_sb[:, sl],
                                         start=True, stop=True)
                        nc.vector.tensor_mul(dst[:, sl], raw[:, sl], bb[:])
                    return f

                # Q/K/V per tq-chunk so attention can start on early chunks.
                # rsqrt = approx-recip(Sqrt): Square/Sqrt share one ACT
                # table set, reciprocal_approx_fast is one DVE pass.
                # Every PE op that consumes a fresh ACT/DVE product (sums
                # after Square, transposes after the VT copy, bb after the
                # recip chain) is deferred >=1 proj-group behind its
                # producer, so phase B has no PE micro-gaps and HAM warms
                # at ~10us instead of ~28us.
                def mk_sums(sq, g2_sb, rms_sb, rrf_sb, rec_sb, raw, dst, sl):
                    # rec = (mean+eps)^-1/2 = exp(-0.5*ln(mean+eps)): stays
                    # inside the pinned exp/ln/square table set (Sqrt would
                    # force a mid-kernel table reload) and needs no DVE pass
                    def f():
                        sums = ps_sums.tile([2, TQ], F32, tag="sums",
                                            name="sums")
                        nc.tensor.matmul(sums[:], onescol_sb[:], sq[:],
                                         start=True, stop=True)
                        nc.scalar.activation(rms_sb[:, sl], sums[:],
                                             AF.Ln, bias=eps_sb[:],
                                             scale=1.0 / D)
                        nc.scalar.activation(rec_sb[:, sl], rms_sb[:, sl],
                                             AF.Exp, scale=-0.5)
                    return f

                def mk_tp(c4):
                    def f():
                        for r in range(4 * c4, 4 * c4 + 4):
                            tp = ps_tp.tile([128, 128], BF16, tag="tp",
                                            name=f"tp{r}")
                            nc.tensor.transpose(
                                tp[:], VT_sb[:, 128 * r:128 * (r + 1)],
                                ident_sb[:])
                            nc.vector.tensor_copy(V_sb[:, r, 0:64],
                                                  tp[:, 0:64])
                            nc.vector.tensor_copy(V_sb[:, r, 65:129],
                                                  tp[:, 64:128])
                    return f

                pend_b = []
                for c4 in range(NTQ):
                    sl = slice(TQ * c4, TQ * (c4 + 1))
                    chains = []
                    for jbase, g2_sb, rms_sb, rrf_sb, rec_sb, raw, dst in (
                        (0, gq_sb, rms_q, rrf_q, rec_q, QT_raw, QTn),
                        (JPC, gk_sb, rms_k, rrf_k, rec_k, KT_raw, KTn),
                    ):
                        pp = ps_proj.tile([128, TQ], F32, tag="proj",
                                          name="pp")
                        for ci in range(8):
                            nc.tensor.matmul(
                                pp[:], wsb[:, ci, jbase:jbase + 128],
                                xT_sb[:, ci, sl],
                                start=(ci == 0), stop=(ci == 7))
                        sq = sqp.tile([128, TQ], BF16, tag="sq", name="sq")
                        nc.scalar.activation(sq[:], pp[:], AF.Square)
                        nc.vector.tensor_copy(raw[:, sl], pp[:])
                        if jbase == 0:  # after Q proj: drain prev chunk
                            while pend_b:
                                pend_b.pop(0)()
                        else:           # after K proj: this chunk's Q sums
                            chains[0]()
                        chains.append(
                            mk_sums(sq, g2_sb, rms_sb, rrf_sb, rec_sb,
                                    raw, dst, sl))
                        pend_b.append(
                            mk_bb(g2_sb, rec_sb, raw, dst, sl))

                    # V^T projection chunk
                    pv = ps_proj.tile([128, TQ], F32, tag="proj", name="pv")
                    for ci in range(8):
                        nc.tensor.matmul(
                            pv[:], wsb[:, ci, 2 * JPC:3 * JPC],
                            xT_sb[:, ci, sl],
                            start=(ci == 0), stop=(ci == 7))
                    chains[1]()   # this chunk's K sums after V proj
                    nc.vector.tensor_copy(VT_sb[:, sl], pv[:])
                    pend_b.append(mk_tp(c4))
                while pend_b:
                    pend_b.pop(0)()

            # ========== Phase C: attention + normalize + w_o, per chunk ====
            outT_r = outT_d.rearrange("(m p) t -> p m t", p=128)
            with (
                tc.tile_pool(name="ep", bufs=6) as ep,
                tc.tile_pool(name="stgp", bufs=1) as stgp,
                tc.tile_pool(name="ps_st0", bufs=1, space="PSUM") as ps_st0,
                tc.tile_pool(name="ps_st1", bufs=1, space="PSUM") as ps_st1,
                tc.tile_pool(name="ps_ot", bufs=1, space="PSUM") as ps_ot,
                tc.tile_pool(name="ps_wrk", bufs=2, space="PSUM") as ps_wrk,
            ):
                st_pools = (ps_st0, ps_st1)
                stg_big = stgp.tile([128, 8, T], BF16)  # w_o out staging

                def mk_pending(p4):
                    """Chunk p4's normalize + w_o, deferred into the next
                    chunk's attention loop as PE fillers: its DVE reciprocal
                    chain gets the chunk boundary to drain, so the PE never
                    waits on it (the b2/w_o stall here re-throttled HAM to
                    1.2GHz for ~17us in the undeferred layout)."""
                    psl = slice(TQ * p4, TQ * (p4 + 1))
                    todo = []

                    def b2fn(h, psl=psl):
                        hsl = slice(64 * h, 64 * (h + 1))
                        b2 = ps_wrk.tile([128, TQ], F32, tag="wrk",
                                         name=f"b2{h}_{p4}")
                        nc.tensor.matmul(b2[:], oneh_sb[h][:],
                                         recs[h][0:1, psl],
                                         start=True, stop=True)
                        nc.vector.tensor_mul(ctxT[hsl, psl],
                                             ctx_un[hsl, psl], b2[hsl, :])

                    def wofn(mu, psl=psl, p4=p4):
                        wop = ps_wrk.tile([128, TQ], F32, tag="wrk",
                                          name=f"wop{mu}_{p4}")
                        nc.tensor.matmul(wop[:],
                                         wo_sb[:, 128 * mu:128 * (mu + 1)],
                                         ctxT[:, psl], start=True, stop=True)
                        nc.vector.tensor_copy(stg_big[:, mu, psl], wop[:])
                        nc.sync.dma_start(out=outT_r[:, mu, psl],
                                          in_=stg_big[:, mu, psl])

                    for h in range(HPC):
                        todo.append(lambda h=h: b2fn(h))
                    for mu in range(8):
                        todo.append(lambda mu=mu: wofn(mu))
                    return todo

                pending = []
                for c4 in range(NTQ):
                    qsl = slice(TQ * c4, TQ * (c4 + 1))
                    n_tk = 4 * (c4 + 1)
                    ot = [ps_ot.tile([65, TQ], F32, tag=f"ot{h}",
                                     name=f"ot{h}_{c4}")
                          for h in range(HPC)]
                    for g in range(2 * c4):   # off-diagonal r-tiles, paired
                        for h in range(HPC):
                            hsl = slice(64 * h, 64 * (h + 1))
                            st = st_pools[h].tile([128, 2 * TQ], F32,
                                                  tag=f"st{h}",
                                                  name=f"st{h}")
                            for rl in range(2):
                                r = 2 * g + rl
                                nc.tensor.matmul(
                                    st[:, TQ * rl:TQ * (rl + 1)],
                                    KTn[hsl, 128 * r:128 * (r + 1)],
                                    QTn[hsl, qsl], start=True, stop=True)
                            e_t = ep.tile([128, 2 * TQ], BF16, tag=f"e{h}",
                                          name=f"e{h}")
                            nc.scalar.activation(e_t[:], st[:], AF.Exp,
                                                 scale=float(D) ** -0.5)
                            for rl in range(2):
                                r = 2 * g + rl
                                nc.tensor.matmul(
                                    ot[h][:],
                                    V_sb[:, r, 65 * h:65 * (h + 1)],
                                    e_t[:, TQ * rl:TQ * (rl + 1)],
                                    start=(r == 0), stop=False)
                            if pending:
                                pending.pop(0)()
                    # ragged diagonal band: r-tile 4*c4+s only covers
                    # tq>=128*s (widths 512/384/256/128); only the four
                    # 128x128 true-diagonal blocks need a triangle mask
                    # (= first 128 cols of msb plane 0)
                    tri = msb[:, 0, 0:128]
                    for h in range(HPC):
                        hsl = slice(64 * h, 64 * (h + 1))
                        r = 4 * c4

                        def kt(s, hsl=hsl, r=r):
                            return KTn[hsl, 128 * (r + s):128 * (r + s + 1)]

                        def qt(s, hsl=hsl, c4=c4):
                            return QTn[hsl, TQ * c4 + 128 * s:TQ * (c4 + 1)]

                        stb = st_pools[h].tile([128, 2 * TQ], F32,
                                               tag=f"st{h}", name=f"stb{h}")
                        sb2 = ps_wrk.tile([128, TQ], F32, tag="wrk",
                                          name=f"sb2{h}")
                        nc.tensor.matmul(stb[:, 0:512], kt(0), qt(0),
                                         start=True, stop=True)
                        nc.tensor.matmul(stb[:, 512:896], kt(1), qt(1),
                                         start=True, stop=True)
                        nc.tensor.matmul(stb[:, 896:1024], kt(3), qt(3),
                                         start=True, stop=True)
                        nc.tensor.matmul(sb2[:, 0:256], kt(2), qt(2),
                                         start=True, stop=True)
                        e1 = ep.tile([128, 2 * TQ], BF16, tag=f"e{h}",
                                     name=f"eb{h}")
                        nc.scalar.activation(e1[:], stb[:], AF.Exp,
                                             scale=float(D) ** -0.5)
                        e2 = ep.tile([128, 256], BF16, tag=f"e2{h}",
                                     name=f"eb2{h}")
                        nc.scalar.activation(e2[:], sb2[:, 0:256], AF.Exp,
                                             scale=float(D) ** -0.5)
                        for blk in (e1[:, 0:128], e1[:, 512:640],
                                    e1[:, 896:1024], e2[:, 0:128]):
                            nc.vector.tensor_mul(blk, blk, tri)
                        vs = V_sb[:, :, 65 * h:65 * (h + 1)]
                        nc.tensor.matmul(ot[h][:, 0:512], vs[:, r],
                                         e1[:, 0:512],
                                         start=(r == 0), stop=False)
                        nc.tensor.matmul(ot[h][:, 128:512], vs[:, r + 1],
                                         e1[:, 512:896],
                                         start=False, stop=False)
                        nc.tensor.matmul(ot[h][:, 256:512], vs[:, r + 2],
                                         e2[:, 0:256],
                                         start=False, stop=False)
                        nc.tensor.matmul(ot[h][:, 384:512], vs[:, r + 3],
                                         e1[:, 896:1024],
                                         start=False, stop=True)
                        if pending:
                            pending.pop(0)()
                    while pending:
                        pending.pop(0)()
                    # stage ctx + softmax sums (approx recip); b2/w_o are
                    # deferred into the next chunk's attention
                    for h in range(HPC):
                        hsl = slice(64 * h, 64 * (h + 1))
                        nc.vector.tensor_copy(ctx_un[hsl, qsl],
                                              ot[h][0:64, :])
                        nc.vector.tensor_copy(sg[h][0:1, qsl],
                                              ot[h][64:65, :])
                        nc.vector.reciprocal_approx_fast(
                            out=sgf[h][0:1, qsl], in_=sg[h][0:1, qsl])
                        nc.vector.tensor_copy(recs[h][0:1, qsl],
                                              sgf[h][0:1, qsl])
                    pending = mk_pending(c4)
                while pending:
                    pending.pop(0)()

    nc.compile()
    return nc


_NC_CACHE = None


def _get_nc():
    global _NC_CACHE
    if _NC_CACHE is None:
        _NC_CACHE = build_nc()
    return _NC_CACHE


def _make_in_maps(x, w_q, w_k, w_v, w_o, q_gamma, k_gamma):
    x = np.asarray(x, dtype=np.float32)
    xT = np.ascontiguousarray(x.reshape(T, C).T).astype(NP_BF16)  # [C, T]

    p = np.arange(128)
    f = np.arange(TQ)
    masks = np.zeros((4, 128, TQ), dtype=NP_BF16)
    for s in range(4):
        masks[s] = (f[None, :] >= (p[:, None] + 128 * s)).astype(NP_BF16)

    blk = (p[None, :] // 64 == np.arange(2)[:, None])      # [2, 128] bool
    gq = blk * np.tile(np.asarray(q_gamma, np.float32), 2)[None, :]
    gk = blk * np.tile(np.asarray(k_gamma, np.float32), 2)[None, :]
    ones2 = blk.astype(NP_BF16)
    onescol = np.ascontiguousarray(ones2.T)
    ident = np.eye(128, dtype=NP_BF16)

    common = dict(xT=xT, masks=masks,
                  gq=gq.astype(NP_BF16), gk=gk.astype(NP_BF16),
                  ones2=ones2, onescol=onescol, ident=ident,
                  vones=np.ones((128, 32), dtype=NP_BF16))

    in_maps = []
    for i in range(NCORES):
        rows = slice(JPC * i, JPC * (i + 1))
        wqkv = np.concatenate(
            [np.asarray(w_q, np.float32)[rows].T,
             np.asarray(w_k, np.float32)[rows].T,
             np.asarray(w_v, np.float32)[rows].T], axis=1)  # [C, 384]
        wo = np.asarray(w_o, np.float32)[:, rows].T          # [128, C]
        in_maps.append(dict(common,
                            wqkv=np.ascontiguousarray(wqkv).astype(NP_BF16),
                            wo=np.ascontiguousarray(wo).astype(NP_BF16)))
    return in_maps


def _run(x, w_q, w_k, w_v, w_o, q_gamma, k_gamma, trace=False):
    import time

    from concourse.bass_utils import run_bass_kernel_spmd
    nc = _get_nc()
    in_maps = _make_in_maps(x, w_q, w_k, w_v, w_o, q_gamma, k_gamma)
    res = None
    for attempt in range(3):
        try:
            res = run_bass_kernel_spmd(nc, in_maps, list(range(NCORES)),
                                       trace=trace)
            break
        except Exception:
            # rare transient NRT_EXEC_UNIT_UNRECOVERABLE under axon; the
            # terminal resets the device on the next load
            if attempt == 2:
                raise
            time.sleep(3.0)
    acc = np.zeros((C, T), dtype=np.float64)
    for r in res.results:
        acc += r["outT"].astype(np.float64)
    out = acc.T.astype(np.float32).reshape(1, T, C)
    return out, res


def kernel(x, w_q, w_k, w_v, w_o, q_gamma, k_gamma):
    out, _ = _run(x, w_q, w_k, w_v, w_o, q_gamma, k_gamma, trace=False)
    return out

